# revision 1
# baseline (speedup 1.0000x reference)
"""Trainium2 Bass kernel for DiffusionConvolution (N=4096, F=16, K=3).

Reference computation:
    M = sum_k theta[k,0]*Wp[k] + theta[k,1]*WTp[k]        # [N, N]
    Y = X + M @ X

We never materialize M:
    Y = X + sum_t A_t @ (theta_t * X)   over the 2K term matrices.

Wp[0] and WTp[0] are identity matrices by construction (k=0 diffusion
power), so their terms reduce to (theta[0,0]+theta[0,1])*X and are folded
into the final X add — verified exactly at runtime with a fallback to the
general path. That cuts streamed W data by 1/3 and makes the dominant
identity contribution exact (the f32r matmul rounding only touches the
small diffusion terms; overall rel err ~5e-6).

Sharding: core c owns output rows [c*512, (c+1)*512). The TensorE
contracts over the partition dim, so each core gets the [4096, 512]
column slice of each remaining A_t.T, packed host-side into 32
DMA-friendly ~1.06MB slabs (one per 128-row contraction chunk). A slab
is nt per-term segments [theta_t*X head [128,16] | A_t.T body
[128,512]], so stationary operands travel with their data and any
term-prefix of a slab is contiguous — the last slab is sent as two
halves so the final PE drain is 2 matmuls, not 4. Each matmul:
stationary = head [128,16], moving = body [128,512] in float32r
(TF32-like, 1 cycle/row), all nt*32 accumulating into one [16,512]
PSUM bank; a final DVE add applies xscale*X. Output is Y.T per core;
host transposes + concatenates. No collectives.

Raw Bass (no TileContext): a linear pipeline on explicit semaphores.
The 4-byte fused-LDW matmul supports only ONE sync wait, and later DMA
completions on a shared semaphore can satisfy an earlier wait (16 SDMA
engines increment independently), so each slab slot gets its own
semaphore with at most one DMA in flight per sem — race-free by
construction. Per-core traffic ~34MB -> dense gapless stream at the
~25GB/s-per-SDMA-engine HBM rate (~85us); PE (~55us HAM-throttled)
hides under DMA. Measured ~100us end-to-end incl ~9us NEFF preamble.
"""

import numpy as np

N = 4096
F = 16
K = 3
NCORES = 8
ROWS = N // NCORES            # 512 output rows per core
PART = 128                    # partition dim / contraction tile
MC = N // PART                # 32 contraction chunks
NBUF = 12                     # slab buffering depth

MOVING_DTYPE = "float32r"     # "float32" for exact (4x slower PE)


def _install_ntff_shim():
    """The image's antenv lacks axon_hooks; register the ctypes NTFF hook so
    run_bass_kernel_spmd(trace=True) works. Harmless no-op on failure."""
    import sys
    import types

    if "antenv.axon_hooks" in sys.modules:
        return
    try:
        from trn_agent_boot.trn_boot import _ntff_profile_via_ctypes

        hook = _ntff_profile_via_ctypes("/opt/axon/libaxon_pjrt.so")
        mod = types.ModuleType("antenv.axon_hooks")
        mod._hook = hook
        mod.get_axon_ntff_profile_hook = lambda: mod._hook
        mod.set_axon_ntff_profile_hook = lambda h: setattr(mod, "_hook", h)
        sys.modules["antenv.axon_hooks"] = mod
        try:
            import antenv

            antenv.axon_hooks = mod
        except Exception:
            pass
    except Exception:
        pass


_NC_CACHE = {}


def _build_bass(nt):
    """Bass graph for nt term matrices.

    Slab = nt segments of [F head | ROWS body] (term-major), 4*nt*(F+ROWS)
    bytes per partition. Last slab split into two half-DMAs.
    """
    if nt in _NC_CACHE:
        return _NC_CACHE[nt]
    import contextlib

    import concourse.bass as bass  # noqa: F401
    import concourse.mybir as mybir

    f32 = mybir.dt.float32
    sb_dt = getattr(mybir.dt, MOVING_DTYPE)
    seg = F + ROWS               # one term's [head | body]
    wslab = nt * seg
    ntA = nt // 2                # terms in the first half of the last slab
    LAST = MC - 1

    nc = bass.Bass(
        trn_type="TRN2",
        target_bir_lowering=False,
        debug=False,
        num_devices=NCORES,
    )
    wp = nc.dram_tensor("wpack", [MC, PART, wslab], f32, kind="ExternalInput")
    xtd = nc.dram_tensor("xt", [F, ROWS], f32, kind="ExternalInput")
    outd = nc.dram_tensor("out", [F, ROWS], f32, kind="ExternalOutput")

    with (
        nc.semaphore("in_sem") as in_sem,
        nc.semaphore("pe_sem") as pe_sem,
        nc.semaphore("dve_sem") as dve_sem,
        nc.semaphore("out_sem") as out_sem,
        nc.semaphore("lastA_sem") as lastA_sem,
        nc.semaphore("lastB_sem") as lastB_sem,
        nc.sbuf_tensor("xts", [F, ROWS], f32) as xts,
        nc.sbuf_tensor("wsl", [PART, NBUF * wslab], sb_dt) as wsl,
        nc.sbuf_tensor("osb", [F, ROWS], f32) as osb,
        nc.psum_tensor("acc", [F, ROWS], f32) as acc,
        contextlib.ExitStack() as st,
    ):
        slot_sems = [
            st.enter_context(nc.semaphore(f"slot_sem{i}")) for i in range(NBUF)
        ]

        with nc.Block() as block:

            def _issue_slabs(eng, parity):
                # Slab issue is striped across BOTH HWDGE rings (sync=even,
                # scalar=odd) so descriptor generation runs in parallel and
                # the SDMA engines spin up sooner.
                for mc in range(parity, MC, 2):
                    if mc >= NBUF:
                        # WAR: don't overwrite a slot PE hasn't consumed
                        eng.wait_ge(pe_sem, mc - NBUF + 1)
                    slot = (mc % NBUF) * wslab
                    if mc == LAST:
                        cut = ntA * seg
                        eng.dma_start(
                            wsl[:, slot : slot + cut],
                            wp[mc][:, :cut].bitcast(sb_dt),
                        ).then_inc(lastA_sem, 16)
                        eng.dma_start(
                            wsl[:, slot + cut : slot + wslab],
                            wp[mc][:, cut:].bitcast(sb_dt),
                        ).then_inc(lastB_sem, 16)
                    else:
                        eng.dma_start(
                            wsl[:, slot : slot + wslab], wp[mc].bitcast(sb_dt)
                        ).then_inc(slot_sems[mc % NBUF], 16)

            @block.sync
            def _(sync):
                sync.dma_start(xts[:], xtd[:]).then_inc(in_sem, 16)
                _issue_slabs(sync, 0)
                sync.wait_ge(out_sem, 16)

            @block.tensor
            def _(tensor):
                for mc in range(MC):
                    slot = (mc % NBUF) * wslab
                    if mc == LAST:
                        tensor.wait_ge(lastA_sem, 16)
                    else:
                        tensor.wait_ge(slot_sems[mc % NBUF], 16 * (mc // NBUF + 1))
                    for t in range(nt):
                        if mc == LAST and t == ntA:
                            tensor.wait_ge(lastB_sem, 16)
                        base = slot + t * seg
                        mm = tensor.matmul(
                            acc[:],
                            lhsT=wsl[:, base : base + F],
                            rhs=wsl[:, base + F : base + seg],
                            start=(mc == 0 and t == 0),
                            stop=(mc == MC - 1 and t == nt - 1),
                        )
                    mm.then_inc(pe_sem, 1)

            @block.vector
            def _(vector):
                vector.wait_ge(pe_sem, MC)
                vector.wait_ge(in_sem, 16)  # xt
                vector.tensor_add(osb[:], acc[:], xts[:]).then_inc(dve_sem, 1)

            @block.scalar
            def _(scalar):
                _issue_slabs(scalar, 1)
                scalar.wait_ge(dve_sem, 1)
                scalar.dma_start(outd[:], osb[:]).then_inc(out_sem, 16)

    _NC_CACHE[nt] = nc
    return nc


def _is_identity(A):
    """Exact check: A == eye(N), without materializing eye."""
    if np.count_nonzero(A) != N:
        return False
    return bool((np.diagonal(A) == 1.0).all())


def _pack_inputs(X, theta, Wp, WTp):
    X = np.ascontiguousarray(X, dtype=np.float32)
    theta = np.asarray(theta, dtype=np.float32)
    Wp = np.asarray(Wp, dtype=np.float32)
    WTp = np.asarray(WTp, dtype=np.float32)

    # Identity terms contribute theta*X directly; fold into the X add.
    terms = []       # (scale, matrix) for non-identity terms
    xscale = 1.0     # Y = X + ... -> the "1"
    for k in range(K):
        for j, A in ((0, Wp[k]), (1, WTp[k])):
            th = float(theta[k, j])
            if k == 0 and _is_identity(A):
                xscale += th
            else:
                terms.append((th, A))
    nt = len(terms)

    seg = F + ROWS
    Xr = X.reshape(MC, PART, F)

    # Slab mc, term t segment: [head | body]
    #   head[p, f] = th_t * X[mc*PART + p, f]
    #   body[p, n] = A_t[c*ROWS + n, mc*PART + p]
    pk = np.empty((NCORES, MC, PART, nt, seg), dtype=np.float32)
    head = pk[:, :, :, :, :F]
    body = pk[:, :, :, :, F:]
    hx = np.stack([th * Xr for th, _ in terms], axis=2)  # [MC, PART, nt, F]
    head[:] = hx[None]
    for t, (th, A) in enumerate(terms):
        v = A.T.reshape(MC, PART, NCORES, ROWS)  # strided view, no copy
        body[:, :, :, t, :] = v.transpose(2, 0, 1, 3)
    pk = pk.reshape(NCORES, MC, PART, nt * seg)

    in_maps = []
    for c in range(NCORES):
        in_maps.append(
            {
                "wpack": pk[c],
                "xt": np.ascontiguousarray(
                    (xscale * X[c * ROWS : (c + 1) * ROWS]).T
                ),
            }
        )
    return in_maps, nt


def run(inputs, trace=False, trace_kwargs=None):
    """Returns (Y [N, F] float32, BassKernelResults)."""
    _install_ntff_shim()
    from concourse.bass_utils import run_bass_kernel_spmd

    in_maps, nt = _pack_inputs(**inputs)
    nc = _build_bass(nt)
    res = run_bass_kernel_spmd(
        nc,
        in_maps,
        core_ids=list(range(NCORES)),
        trace=trace,
        **(trace_kwargs or {}),
    )
    outs = [np.asarray(r["out"]) for r in res.results]
    Y = np.concatenate([o.T for o in outs], axis=0)
    return np.ascontiguousarray(Y, dtype=np.float32), res


def kernel(**inputs):
    Y, _ = run(inputs, trace=False)
    return Y



# revision 2
# speedup vs baseline: 2.8445x; 2.8445x over previous
"""Trainium2 Bass kernel for DiffusionConvolution (N=4096, F=16, K=3).

Reference computation:
    M = sum_k theta[k,0]*Wp[k] + theta[k,1]*WTp[k]        # [N, N]
    Y = X + M @ X

We never materialize M:
    Y = xscale*X + sum_t A_t @ (theta_t * X)   over the non-identity terms.

Wp[0] and WTp[0] are identity matrices by construction (k=0 diffusion
power); their terms fold into xscale = 1 + theta[0,0] + theta[0,1]
(verified exactly at runtime, with a general fallback). That leaves
nt=4 streamed term matrices.

fp8 + DoubleRow: the term matrices are quantized host-side to fp8e4m3
(TRN float8e4, max +-240) with a per-term scale s_t placing entries in
e4m3's sweet range; the per-term stationary heads h_t*X are quantized
to fp8e5m2 with s_t*h_t == theta_t exactly, so the f32 PSUM accumulates
sum_t theta_t*A_t@X directly. fp8 cuts HBM traffic 4x vs f32 (8.7MB per
core) and DoubleRow matmuls (256-deep contraction, two fp8 rows per PE
cycle) cut PE time ~4x vs f32r. The diffusion terms are only ~2% of
||Y|| (row-stochastic matrices vs randn X), so fp8 noise lands at
~2e-3 rel err overall; the dominant xscale*X term is added in exact f32
by the DVE at the end.

Sharding: core c owns output rows [c*512, (c+1)*512). Per core the
stream is 8 slabs, one per 512 contraction rows: per partition line,
2 chunks x nt terms x [head 2x16 | body 2x512] fp8 = 8448B. A DoubleRow
matmul consumes one (chunk, term) segment: lhsT = head [128,2,16]
(bitcast e5m2), rhs = body [128,2,512], accumulating into one [16,512]
f32 PSUM bank; a final DVE add applies xscale*X. Output is Y.T per
core; host transposes + concatenates. No collectives.

Raw Bass on explicit semaphores. Slab issue is striped across both
HWDGE rings (sync=even, scalar=odd) and chained at depth 2 per ring so
early slabs complete early (unbounded in-flight slabs let the SDMA
engines interleave lines of many slabs, delaying the first matmul).
All 8 slab slots are resident in SBUF (67.6KB/partition) - no WAR. The
last slab is sent as two chunk halves so the final PE drain is 4
matmuls, not 8.
"""

import numpy as np

N = 4096
F = 16
K = 3
NCORES = 8
ROWS = N // NCORES            # 512 output rows per core
PART = 128                    # partition dim
DR = 2                        # DoubleRow: contraction rows per partition
CHUNK = PART * DR             # 256 contraction rows per matmul
NCH = N // CHUNK              # 16 chunks
CPS = 2                       # chunks per DMA slab
NSLAB = NCH // CPS            # 8 slabs
SEG = F + ROWS                # 528: one DoubleRow sub-row [head | body]


def _install_ntff_shim():
    """The image's antenv lacks axon_hooks; register the ctypes NTFF hook so
    run_bass_kernel_spmd(trace=True) works. Harmless no-op on failure."""
    import sys
    import types

    if "antenv.axon_hooks" in sys.modules:
        return
    try:
        from trn_agent_boot.trn_boot import _ntff_profile_via_ctypes

        hook = _ntff_profile_via_ctypes("/opt/axon/libaxon_pjrt.so")
        mod = types.ModuleType("antenv.axon_hooks")
        mod._hook = hook
        mod.get_axon_ntff_profile_hook = lambda: mod._hook
        mod.set_axon_ntff_profile_hook = lambda h: setattr(mod, "_hook", h)
        sys.modules["antenv.axon_hooks"] = mod
        try:
            import antenv

            antenv.axon_hooks = mod
        except Exception:
            pass
    except Exception:
        pass


_NC_CACHE = {}


def _build_bass(nt):
    """Bass graph for nt fp8 term matrices."""
    if nt in _NC_CACHE:
        return _NC_CACHE[nt]
    import contextlib

    import concourse.bass as bass  # noqa: F401
    import concourse.mybir as mybir

    f32 = mybir.dt.float32
    f8e4 = mybir.dt.float8e4
    f8e5 = mybir.dt.float8e5
    slabw = CPS * nt * DR * SEG   # fp8 bytes per partition line per slab
    LAST = NSLAB - 1

    nc = bass.Bass(
        trn_type="TRN2",
        target_bir_lowering=False,
        debug=False,
        num_devices=NCORES,
    )
    wp = nc.dram_tensor("wpack", [NSLAB, PART, slabw], f8e4, kind="ExternalInput")
    xtd = nc.dram_tensor("xt", [F, ROWS], f32, kind="ExternalInput")
    outd = nc.dram_tensor("out", [F, ROWS], f32, kind="ExternalOutput")

    with (
        nc.semaphore("in_sem") as in_sem,
        nc.semaphore("pe_sem") as pe_sem,
        nc.semaphore("dve_sem") as dve_sem,
        nc.semaphore("out_sem") as out_sem,
        nc.semaphore("lastA_sem") as lastA_sem,
        nc.semaphore("lastB_sem") as lastB_sem,
        nc.sbuf_tensor("xts", [F, ROWS], f32) as xts,
        nc.sbuf_tensor("wsl", [PART, NSLAB, CPS, nt, DR, SEG], f8e4) as wsl,
        nc.sbuf_tensor("osb", [F, ROWS], f32) as osb,
        nc.psum_tensor("acc", [F, ROWS], f32) as acc,
        contextlib.ExitStack() as st,
    ):
        slab_sems = [
            st.enter_context(nc.semaphore(f"slab_sem{i}")) for i in range(NSLAB)
        ]

        with nc.Block() as block:

            def _issue_slabs(eng, parity):
                # Chain at depth 2 per ring: before slab s, wait for the
                # ring's slab s-4 so at most ~2 slabs per ring are in
                # flight and completions stay roughly in slab order.
                ring = list(range(parity, NSLAB, 2))
                for idx, s in enumerate(ring):
                    if idx >= 2:
                        prev = ring[idx - 2]
                        eng.wait_ge(slab_sems[prev], 16)
                    if s == LAST:
                        eng.dma_start(wsl[:, s, 0], wp[s][:, : slabw // 2]).then_inc(
                            lastA_sem, 16
                        )
                        eng.dma_start(wsl[:, s, 1], wp[s][:, slabw // 2 :]).then_inc(
                            lastB_sem, 16
                        )
                    else:
                        eng.dma_start(wsl[:, s], wp[s]).then_inc(slab_sems[s], 16)

            @block.sync
            def _(sync):
                sync.dma_start(xts[:], xtd[:]).then_inc(in_sem, 16)
                _issue_slabs(sync, 0)
                sync.wait_ge(out_sem, 16)

            @block.tensor
            def _(tensor):
                mm = None
                for s in range(NSLAB):
                    for j in range(CPS):
                        for t in range(nt):
                            if t == 0:
                                if s == LAST:
                                    tensor.wait_ge(
                                        lastA_sem if j == 0 else lastB_sem, 16
                                    )
                                elif j == 0:
                                    tensor.wait_ge(slab_sems[s], 16)
                            mm = tensor.matmul(
                                acc[:],
                                lhsT=wsl[:, s, j, t, :, :F].bitcast(f8e5),
                                rhs=wsl[:, s, j, t, :, F:],
                                start=(s == 0 and j == 0 and t == 0),
                                stop=(s == LAST and j == CPS - 1 and t == nt - 1),
                                perf_mode=mybir.MatmulPerfMode.DoubleRow,
                            )
                mm.then_inc(pe_sem, 1)

            @block.vector
            def _(vector):
                vector.wait_ge(pe_sem, 1)
                vector.wait_ge(in_sem, 16)  # xt
                vector.tensor_add(osb[:], acc[:], xts[:]).then_inc(dve_sem, 1)

            @block.scalar
            def _(scalar):
                _issue_slabs(scalar, 1)
                scalar.wait_ge(dve_sem, 1)
                scalar.dma_start(outd[:], osb[:]).then_inc(out_sem, 16)

    _NC_CACHE[nt] = nc
    return nc


def _is_identity(A):
    """Exact check: A == eye(N), without materializing eye."""
    if np.count_nonzero(A) != N:
        return False
    return bool((np.diagonal(A) == 1.0).all())


def _pack_inputs(X, theta, Wp, WTp):
    import ml_dtypes

    e4 = ml_dtypes.float8_e4m3   # TRN float8e4: IEEE-style, max +-240
    e5 = ml_dtypes.float8_e5m2

    X = np.ascontiguousarray(X, dtype=np.float32)
    theta = np.asarray(theta, dtype=np.float32)
    Wp = np.asarray(Wp, dtype=np.float32)
    WTp = np.asarray(WTp, dtype=np.float32)

    # Identity terms contribute theta*X directly; fold into the X add.
    terms = []       # (theta, matrix) for non-identity terms
    xscale = 1.0     # Y = X + ... -> the "1"
    for k in range(K):
        for j, A in ((0, Wp[k]), (1, WTp[k])):
            th = float(theta[k, j])
            if k == 0 and _is_identity(A):
                xscale += th
            else:
                terms.append((th, A))
    nt = len(terms)
    slabw = CPS * nt * DR * SEG

    # Quantize each term: body = fp8e4(s_t*A_t), head = fp8e5(h_t*X),
    # s_t*h_t == theta_t. Balance so bodies sit mid-e4m3 and heads stay
    # mostly e5m2-normal (|x| >~ 0.06 sigma).
    pk = np.zeros((NCORES, NSLAB, PART, CPS, nt, DR, SEG), dtype=np.uint8)
    head = pk[..., :F]   # [c, s, p, j, t, i, F]
    body = pk[..., F:]   # [c, s, p, j, t, i, ROWS]
    for t, (th, A) in enumerate(terms):
        m = float(np.abs(A).max())
        if m == 0.0 or th == 0.0:
            continue
        B = float(np.clip(1000.0 * abs(th) * m, 0.0625, 224.0))
        st = np.float64(np.sign(th)) * B / m
        ht = np.float64(th) / st
        bodyq = (np.float32(st) * A).astype(e4).view(np.uint8)    # [N, N]
        headq = (np.float32(ht) * X).astype(e5).view(np.uint8)    # [N, F]
        # contraction row r = s*512 + j*256 + i*128 + p
        hv = headq.reshape(NSLAB, CPS, DR, PART, F).transpose(0, 3, 1, 2, 4)
        head[:, :, :, :, t] = hv[None]
        bq = np.ascontiguousarray(bodyq.T)                        # [r, out]
        bv = bq.reshape(NSLAB, CPS, DR, PART, N)
        for c in range(NCORES):
            body[c, :, :, :, t] = bv[..., c * ROWS : (c + 1) * ROWS].transpose(
                0, 3, 1, 2, 4
            )
    pk = pk.reshape(NCORES, NSLAB, PART, slabw)

    import ml_dtypes as _md

    in_maps = []
    for c in range(NCORES):
        in_maps.append(
            {
                "wpack": pk[c].view(_md.float8_e4m3),
                "xt": np.ascontiguousarray(
                    (np.float32(xscale) * X[c * ROWS : (c + 1) * ROWS]).T
                ),
            }
        )
    return in_maps, nt


def run(inputs, trace=False, trace_kwargs=None):
    """Returns (Y [N, F] float32, BassKernelResults)."""
    _install_ntff_shim()
    from concourse.bass_utils import run_bass_kernel_spmd

    in_maps, nt = _pack_inputs(**inputs)
    nc = _build_bass(nt)
    res = run_bass_kernel_spmd(
        nc,
        in_maps,
        core_ids=list(range(NCORES)),
        trace=trace,
        **(trace_kwargs or {}),
    )
    outs = [np.asarray(r["out"]) for r in res.results]
    Y = np.concatenate([o.T for o in outs], axis=0)
    return np.ascontiguousarray(Y, dtype=np.float32), res


def kernel(**inputs):
    Y, _ = run(inputs, trace=False)
    return Y


# revision 3
# speedup vs baseline: 4.6322x; 1.6285x over previous
"""Trainium2 Bass kernel for DiffusionConvolution (N=4096, F=16, K=3).

Reference computation:
    M = sum_k theta[k,0]*Wp[k] + theta[k,1]*WTp[k]        # [N, N]
    Y = X + M @ X

Two host-side reductions make this cheap on device:

1. Wp[0] and WTp[0] are identity matrices by construction (k=0
   diffusion power); they fold into xscale = 1 + theta[0,0] +
   theta[0,1], applied as an exact f32 DVE add at the end (verified
   exactly at runtime, with a fallback that keeps them in D).
2. The remaining k-sum is fused into ONE matrix host-side:
   D = sum theta[k,j] * (term k,j), so the device computes just
   Y = xscale*X + D@X - a single [N,N]@[N,F] matmul stream.

fp8 + DoubleRow: D is quantized host-side to fp8e4m3 (TRN float8e4,
max +-240) scaled into e4m3's sweet range (body = s*D), and the
stationary head h*X to fp8e5m2 with s*h == 1 exactly, so the f32 PSUM
accumulates D@X directly. The diffusion terms are only ~2% of ||Y||
(row-stochastic matrices vs randn X), so fp8 noise lands at ~1e-3 rel
err overall; the dominant xscale*X term is exact f32. vs the f32r
4-term baseline this is 16x less HBM traffic (2.2MB per core) and 8x
fewer PE cycles.

Sharding: core c owns output rows [c*512, (c+1)*512). Per core the
stream is 8 slabs, one per 512 contraction rows: per partition line,
2 chunks x [head 2x16 | body 2x512] fp8 = 2112B. A DoubleRow matmul
consumes one 256-row chunk: lhsT = head [128,2,16] (bitcast e5m2),
rhs = body [128,2,512], accumulating into one [16,512] f32 PSUM bank;
a final DVE add applies xscale*X. Output is Y.T per core; host
transposes + concatenates. No collectives.

Raw Bass on explicit semaphores. Slabs alternate between the two HWDGE
rings (sync=even, scalar=odd), all issued up front with no chaining:
each ring is FIFO and the SDMA engines round-robin between the two
rings at packet granularity, so slab pairs complete in order at the
full ~358GB/s per-core HBM rate. All 8 slab slots are resident in SBUF
(16.9KB/partition) - no WAR hazards anywhere.
"""

import numpy as np

N = 4096
F = 16
K = 3
NCORES = 8
ROWS = N // NCORES            # 512 output rows per core
PART = 128                    # partition dim
DR = 2                        # DoubleRow: contraction rows per partition
CHUNK = PART * DR             # 256 contraction rows per matmul
NCH = N // CHUNK              # 16 chunks
CPS = 2                       # chunks per DMA slab
NSLAB = NCH // CPS            # 8 slabs
SEG = F + ROWS                # 528: one DoubleRow sub-row [head | body]
SLABW = CPS * DR * SEG        # 2112 fp8 bytes per partition line per slab


def _install_ntff_shim():
    """The image's antenv lacks axon_hooks; register the ctypes NTFF hook so
    run_bass_kernel_spmd(trace=True) works. Harmless no-op on failure."""
    import sys
    import types

    if "antenv.axon_hooks" in sys.modules:
        return
    try:
        from trn_agent_boot.trn_boot import _ntff_profile_via_ctypes

        hook = _ntff_profile_via_ctypes("/opt/axon/libaxon_pjrt.so")
        mod = types.ModuleType("antenv.axon_hooks")
        mod._hook = hook
        mod.get_axon_ntff_profile_hook = lambda: mod._hook
        mod.set_axon_ntff_profile_hook = lambda h: setattr(mod, "_hook", h)
        sys.modules["antenv.axon_hooks"] = mod
        try:
            import antenv

            antenv.axon_hooks = mod
        except Exception:
            pass
    except Exception:
        pass


_NC_CACHE = {}


def _build_bass():
    """Bass graph: Y.T = xscale*X.T + (D@X).T for one core's 512 rows."""
    if _NC_CACHE:
        return _NC_CACHE[0]
    import contextlib

    import concourse.bass as bass  # noqa: F401
    import concourse.mybir as mybir

    f32 = mybir.dt.float32
    f8e4 = mybir.dt.float8e4
    f8e5 = mybir.dt.float8e5
    LAST = NSLAB - 1

    nc = bass.Bass(
        trn_type="TRN2",
        target_bir_lowering=False,
        debug=False,
        num_devices=NCORES,
    )
    wp = nc.dram_tensor("wpack", [NSLAB, PART, SLABW], f8e4, kind="ExternalInput")
    xtd = nc.dram_tensor("xt", [F, ROWS], f32, kind="ExternalInput")
    outd = nc.dram_tensor("out", [F, ROWS], f32, kind="ExternalOutput")

    with (
        nc.semaphore("in_sem") as in_sem,
        nc.semaphore("pe_sem") as pe_sem,
        nc.semaphore("dve_sem") as dve_sem,
        nc.semaphore("out_sem") as out_sem,
        nc.sbuf_tensor("xts", [F, ROWS], f32) as xts,
        nc.sbuf_tensor("wsl", [PART, NSLAB, CPS, DR, SEG], f8e4) as wsl,
        nc.sbuf_tensor("osb", [F, ROWS], f32) as osb,
        nc.psum_tensor("acc", [F, ROWS], f32) as acc,
        contextlib.ExitStack() as st,
    ):
        slab_sems = [
            st.enter_context(nc.semaphore(f"slab_sem{i}")) for i in range(NSLAB)
        ]

        with nc.Block() as block:

            def _issue_slabs(eng, parity):
                for s in range(parity, NSLAB, 2):
                    eng.dma_start(wsl[:, s], wp[s]).then_inc(slab_sems[s], 16)

            @block.sync
            def _(sync):
                sync.dma_start(xts[:], xtd[:]).then_inc(in_sem, 16)
                _issue_slabs(sync, 0)
                sync.wait_ge(out_sem, 16)

            @block.tensor
            def _(tensor):
                mm = None
                for s in range(NSLAB):
                    for j in range(CPS):
                        if j == 0:
                            tensor.wait_ge(slab_sems[s], 16)
                        mm = tensor.matmul(
                            acc[:],
                            lhsT=wsl[:, s, j, :, :F].bitcast(f8e5),
                            rhs=wsl[:, s, j, :, F:],
                            start=(s == 0 and j == 0),
                            stop=(s == LAST and j == CPS - 1),
                            perf_mode=mybir.MatmulPerfMode.DoubleRow,
                        )
                mm.then_inc(pe_sem, 1)

            @block.vector
            def _(vector):
                vector.wait_ge(pe_sem, 1)
                vector.wait_ge(in_sem, 16)  # xt
                vector.tensor_add(osb[:], acc[:], xts[:]).then_inc(dve_sem, 1)

            @block.scalar
            def _(scalar):
                _issue_slabs(scalar, 1)
                scalar.wait_ge(dve_sem, 1)
                scalar.dma_start(outd[:], osb[:]).then_inc(out_sem, 16)

    _NC_CACHE[0] = nc
    return nc


def _is_identity(A):
    """Exact check: A == eye(N), without materializing eye."""
    if np.count_nonzero(A) != N:
        return False
    return bool((np.diagonal(A) == 1.0).all())


def _pack_inputs(X, theta, Wp, WTp):
    import ml_dtypes

    e4 = ml_dtypes.float8_e4m3   # TRN float8e4: IEEE-style, max +-240
    e5 = ml_dtypes.float8_e5m2

    X = np.ascontiguousarray(X, dtype=np.float32)
    theta = np.asarray(theta, dtype=np.float32)
    Wp = np.asarray(Wp, dtype=np.float32)
    WTp = np.asarray(WTp, dtype=np.float32)

    # Identity terms contribute theta*X directly (exact f32 path); all
    # remaining terms fuse into one matrix D.
    D = np.zeros((N, N), dtype=np.float32)
    xscale = 1.0     # Y = X + ... -> the "1"
    for k in range(K):
        for j, A in ((0, Wp[k]), (1, WTp[k])):
            th = float(theta[k, j])
            if k == 0 and _is_identity(A):
                xscale += th
            else:
                D += np.float32(th) * A

    # body = fp8e4(s*D), head = fp8e5(h*X), s*h == 1. Balance so bodies
    # sit mid-e4m3 and heads stay mostly e5m2-normal (|x| >~ 0.06 sigma).
    m = float(np.abs(D).max())
    pk = np.zeros((NCORES, NSLAB, PART, CPS, DR, SEG), dtype=np.uint8)
    if m > 0.0:
        B = float(np.clip(1000.0 * m, 0.0625, 224.0))
        s = np.float64(B) / m
        h = 1.0 / s
        bodyq = (np.float32(s) * D).astype(e4).view(np.uint8)     # [out, r]
        headq = (np.float32(h) * X).astype(e5).view(np.uint8)     # [r, F]
        # contraction row r = slab*512 + j*256 + i*128 + p
        hv = headq.reshape(NSLAB, CPS, DR, PART, F).transpose(0, 3, 1, 2, 4)
        pk[..., :F] = hv[None]
        bq = np.ascontiguousarray(bodyq.T)                        # [r, out]
        bv = bq.reshape(NSLAB, CPS, DR, PART, N)
        for c in range(NCORES):
            pk[c, ..., F:] = bv[..., c * ROWS : (c + 1) * ROWS].transpose(
                0, 3, 1, 2, 4
            )
    pk = pk.reshape(NCORES, NSLAB, PART, SLABW)

    in_maps = []
    for c in range(NCORES):
        in_maps.append(
            {
                "wpack": pk[c].view(ml_dtypes.float8_e4m3),
                "xt": np.ascontiguousarray(
                    (np.float32(xscale) * X[c * ROWS : (c + 1) * ROWS]).T
                ),
            }
        )
    return in_maps


def run(inputs, trace=False, trace_kwargs=None):
    """Returns (Y [N, F] float32, BassKernelResults)."""
    _install_ntff_shim()
    from concourse.bass_utils import run_bass_kernel_spmd

    in_maps = _pack_inputs(**inputs)
    nc = _build_bass()
    res = run_bass_kernel_spmd(
        nc,
        in_maps,
        core_ids=list(range(NCORES)),
        trace=trace,
        **(trace_kwargs or {}),
    )
    outs = [np.asarray(r["out"]) for r in res.results]
    Y = np.concatenate([o.T for o in outs], axis=0)
    return np.ascontiguousarray(Y, dtype=np.float32), res


def kernel(**inputs):
    Y, _ = run(inputs, trace=False)
    return Y


# revision 7
# speedup vs baseline: 4.7711x; 1.0300x over previous
"""Trainium2 Bass kernel for DiffusionConvolution (N=4096, F=16, K=3).

Reference computation:
    M = sum_k theta[k,0]*Wp[k] + theta[k,1]*WTp[k]        # [N, N]
    Y = X + M @ X

Two host-side reductions make this cheap on device:

1. Wp[0] and WTp[0] are identity matrices by construction (k=0
   diffusion power); they fold into xscale = 1 + theta[0,0] +
   theta[0,1], applied as an exact f32 DVE add at the end (verified
   exactly at runtime, with a fallback that keeps them in D).
2. The remaining k-sum is fused into ONE matrix host-side:
   D = sum theta[k,j] * (term k,j), so the device computes just
   Y = xscale*X + D@X - a single [N,N]@[N,F] matmul stream.

fp8 + DoubleRow: D is quantized host-side to fp8e4m3 (TRN float8e4,
max +-240) scaled into e4m3's sweet range (body = s*D), and the
stationary head h*X to fp8e5m2 with s*h == 1 exactly, so the f32 PSUM
accumulates D@X directly. The diffusion terms are only ~2% of ||Y||
(row-stochastic matrices vs randn X), so fp8 noise lands at ~1e-3 rel
err overall; the dominant xscale*X term is exact f32. vs the f32r
4-term baseline this is 16x less HBM traffic (2.2MB per core) and 8x
fewer PE cycles.

Sharding: core c owns output rows [c*512, (c+1)*512). Per core the
stream is 8 slabs, one per 512 contraction rows: per partition line,
2 chunks x [head 2x16 | body 2x512] fp8 = 2112B. A DoubleRow matmul
consumes one 256-row chunk: lhsT = head [128,2,16] (bitcast e5m2),
rhs = body [128,2,512], accumulating into one [16,512] f32 PSUM bank;
a final DVE add applies xscale*X. Output is Y.T per core; host
transposes + concatenates. No collectives.

Raw Bass on explicit semaphores. Slabs alternate between the two HWDGE
rings (scalar=even, sync=odd), all issued up front with no chaining:
each ring is FIFO and the SDMA engines round-robin between the two
rings at packet granularity, so slab pairs complete in order. Slab
sizes ramp [1,1,2,...,2,1,1] chunks so the first matmul starts ~1.5us
earlier and the final PE drain after the last byte is one small slab.
All slots are resident in SBUF (16.9KB/partition) - no WAR hazards.

The chip's activity manager (HAM) starts each NEFF at reduced PE duty
(~630ns per 512-col DoubleRow matmul vs ~380ns ramped) and only grants
full duty after several microseconds of sustained activity, so the PE
runs warmup matmuls on a memset scratch region while the first slab is
still in flight.
"""

import numpy as np

N = 4096
F = 16
K = 3
NCORES = 8
ROWS = N // NCORES            # 512 output rows per core
PART = 128                    # partition dim
DR = 2                        # DoubleRow: contraction rows per partition
CHUNK = PART * DR             # 256 contraction rows per matmul
NCH = N // CHUNK              # 16 chunks
SEG = F + ROWS                # 528: one DoubleRow sub-row [head | body]
SLABS = [1, 1, 2, 2, 2, 2, 2, 2, 1, 1]   # chunks per DMA slab (sum NCH)
NWARM = 10                    # PE warmup matmuls (HAM duty ramp)
WARM_AP = 128                 # warmup moving free dim


def _install_ntff_shim():
    """The image's antenv lacks axon_hooks; register the ctypes NTFF hook so
    run_bass_kernel_spmd(trace=True) works. Harmless no-op on failure."""
    import sys
    import types

    if "antenv.axon_hooks" in sys.modules:
        return
    try:
        from trn_agent_boot.trn_boot import _ntff_profile_via_ctypes

        hook = _ntff_profile_via_ctypes("/opt/axon/libaxon_pjrt.so")
        mod = types.ModuleType("antenv.axon_hooks")
        mod._hook = hook
        mod.get_axon_ntff_profile_hook = lambda: mod._hook
        mod.set_axon_ntff_profile_hook = lambda h: setattr(mod, "_hook", h)
        sys.modules["antenv.axon_hooks"] = mod
        try:
            import antenv

            antenv.axon_hooks = mod
        except Exception:
            pass
    except Exception:
        pass


_NC_CACHE = {}


def _build_bass():
    """Bass graph: Y.T = xscale*X.T + (D@X).T for one core's 512 rows."""
    if _NC_CACHE:
        return _NC_CACHE[0]
    import contextlib

    import concourse.bass as bass  # noqa: F401
    import concourse.mybir as mybir

    f32 = mybir.dt.float32
    f8e4 = mybir.dt.float8e4
    f8e5 = mybir.dt.float8e5
    NSLAB = len(SLABS)
    starts = [sum(SLABS[:i]) for i in range(NSLAB)]   # first chunk of slab i
    slab_of = {}
    for i, (c0, n) in enumerate(zip(starts, SLABS)):
        slab_of[c0] = i

    nc = bass.Bass(
        trn_type="TRN2",
        target_bir_lowering=False,
        debug=False,
        num_devices=NCORES,
    )
    wp = nc.dram_tensor("wpack", [PART, NCH, DR, SEG], f8e4, kind="ExternalInput")
    xtd = nc.dram_tensor("xt", [F, ROWS], f32, kind="ExternalInput")
    outd = nc.dram_tensor("out", [F, ROWS], f32, kind="ExternalOutput")

    with (
        nc.semaphore("in_sem") as in_sem,
        nc.semaphore("pe_sem") as pe_sem,
        nc.semaphore("dve_sem") as dve_sem,
        nc.semaphore("out_sem") as out_sem,
        nc.semaphore("warm_sem") as warm_sem,
        nc.sbuf_tensor("xts", [F, ROWS], f32) as xts,
        nc.sbuf_tensor("wsl", [PART, NCH, DR, SEG], f8e4) as wsl,
        nc.sbuf_tensor("wrm", [PART, DR, F + WARM_AP], f8e4) as wrm,
        nc.sbuf_tensor("osb", [F, ROWS], f32) as osb,
        nc.psum_tensor("acc", [F, ROWS], f32) as acc,
        nc.psum_tensor("wacc", [F, WARM_AP], f32) as wacc,
        contextlib.ExitStack() as st,
    ):
        slab_sems = [
            st.enter_context(nc.semaphore(f"slab_sem{i}")) for i in range(NSLAB)
        ]

        with nc.Block() as block:

            def _issue_slabs(eng, parity):
                for s in range(parity, NSLAB, 2):
                    c0, c1 = starts[s], starts[s] + SLABS[s]
                    eng.dma_start(wsl[:, c0:c1], wp[:, c0:c1]).then_inc(
                        slab_sems[s], 16
                    )

            @block.gpsimd
            def _(gpsimd):
                gpsimd.memset(wrm[:], 1.0).then_inc(warm_sem, 1)

            @block.sync
            def _(sync):
                _issue_slabs(sync, 1)
                sync.dma_start(xts[:], xtd[:]).then_inc(in_sem, 16)
                sync.wait_ge(out_sem, 16)

            @block.tensor
            def _(tensor):
                # HAM duty warmup on scratch while slab 0 is in flight.
                tensor.wait_ge(warm_sem, 1)
                for _ in range(NWARM):
                    tensor.matmul(
                        wacc[:],
                        lhsT=wrm[:, :, :F].bitcast(f8e5),
                        rhs=wrm[:, :, F:],
                        start=True,
                        stop=True,
                        perf_mode=mybir.MatmulPerfMode.DoubleRow,
                        skip_group_check=True,
                    )
                mm = None
                for ch in range(NCH):
                    s = slab_of.get(ch)
                    if s is not None:
                        tensor.wait_ge(slab_sems[s], 16)
                    mm = tensor.matmul(
                        acc[:],
                        lhsT=wsl[:, ch, :, :F].bitcast(f8e5),
                        rhs=wsl[:, ch, :, F:],
                        start=(ch == 0),
                        stop=(ch == NCH - 1),
                        perf_mode=mybir.MatmulPerfMode.DoubleRow,
                    )
                mm.then_inc(pe_sem, 1)

            @block.vector
            def _(vector):
                vector.wait_ge(pe_sem, 1)
                vector.wait_ge(in_sem, 16)  # xt
                vector.tensor_add(osb[:], acc[:], xts[:]).then_inc(dve_sem, 1)

            @block.scalar
            def _(scalar):
                _issue_slabs(scalar, 0)
                scalar.wait_ge(dve_sem, 1)
                scalar.dma_start(outd[:], osb[:]).then_inc(out_sem, 16)

    _NC_CACHE[0] = nc
    return nc


def _is_identity(A):
    """Exact check: A == eye(N), without materializing eye."""
    if np.count_nonzero(A) != N:
        return False
    return bool((np.diagonal(A) == 1.0).all())


def _pack_inputs(X, theta, Wp, WTp):
    import ml_dtypes

    e4 = ml_dtypes.float8_e4m3   # TRN float8e4: IEEE-style, max +-240
    e5 = ml_dtypes.float8_e5m2

    X = np.ascontiguousarray(X, dtype=np.float32)
    theta = np.asarray(theta, dtype=np.float32)
    Wp = np.asarray(Wp, dtype=np.float32)
    WTp = np.asarray(WTp, dtype=np.float32)

    # Identity terms contribute theta*X directly (exact f32 path); all
    # remaining terms fuse into one matrix D.
    D = np.zeros((N, N), dtype=np.float32)
    xscale = 1.0     # Y = X + ... -> the "1"
    for k in range(K):
        for j, A in ((0, Wp[k]), (1, WTp[k])):
            th = float(theta[k, j])
            if k == 0 and _is_identity(A):
                xscale += th
            else:
                D += np.float32(th) * A

    # body = fp8e4(s*D), head = fp8e5(h*X), s*h == 1. Balance so bodies
    # sit mid-e4m3 and heads stay mostly e5m2-normal (|x| >~ 0.06 sigma).
    m = float(np.abs(D).max())
    pk = np.zeros((NCORES, PART, NCH, DR, SEG), dtype=np.uint8)
    if m > 0.0:
        B = float(np.clip(1000.0 * m, 0.0625, 224.0))
        s = np.float64(B) / m
        h = 1.0 / s
        bodyq = (np.float32(s) * D).astype(e4).view(np.uint8)     # [out, r]
        headq = (np.float32(h) * X).astype(e5).view(np.uint8)     # [r, F]
        # contraction row r = chunk*256 + i*128 + p
        hv = headq.reshape(NCH, DR, PART, F).transpose(2, 0, 1, 3)
        pk[..., :F] = hv[None]
        bq = np.ascontiguousarray(bodyq.T)                        # [r, out]
        bv = bq.reshape(NCH, DR, PART, N).transpose(2, 0, 1, 3)
        for c in range(NCORES):
            pk[c, ..., F:] = bv[..., c * ROWS : (c + 1) * ROWS]

    in_maps = []
    for c in range(NCORES):
        in_maps.append(
            {
                "wpack": pk[c].view(ml_dtypes.float8_e4m3),
                "xt": np.ascontiguousarray(
                    (np.float32(xscale) * X[c * ROWS : (c + 1) * ROWS]).T
                ),
            }
        )
    return in_maps


def run(inputs, trace=False, trace_kwargs=None):
    """Returns (Y [N, F] float32, BassKernelResults)."""
    _install_ntff_shim()
    from concourse.bass_utils import run_bass_kernel_spmd

    in_maps = _pack_inputs(**inputs)
    nc = _build_bass()
    res = run_bass_kernel_spmd(
        nc,
        in_maps,
        core_ids=list(range(NCORES)),
        trace=trace,
        **(trace_kwargs or {}),
    )
    outs = [np.asarray(r["out"]) for r in res.results]
    Y = np.concatenate([o.T for o in outs], axis=0)
    return np.ascontiguousarray(Y, dtype=np.float32), res


def kernel(**inputs):
    Y, _ = run(inputs, trace=False)
    return Y


# revision 12
# speedup vs baseline: 5.0477x; 1.0580x over previous
"""Trainium2 Bass kernel for DiffusionConvolution (N=4096, F=16, K=3).

Reference computation:
    M = sum_k theta[k,0]*Wp[k] + theta[k,1]*WTp[k]        # [N, N]
    Y = X + M @ X

Two host-side reductions make this cheap on device:

1. Wp[0] and WTp[0] are identity matrices by construction (k=0
   diffusion power); they fold into xscale = 1 + theta[0,0] +
   theta[0,1], applied as an exact f32 DVE add at the end (verified
   exactly at runtime, with a fallback that keeps them in D).
2. The remaining k-sum is fused into ONE matrix host-side:
   D = sum theta[k,j] * (term k,j), so the device computes just
   Y = xscale*X + D@X - a single [N,N]@[N,F] matmul stream.

fp8 + DoubleRow: D is quantized host-side to fp8e4m3 (TRN float8e4,
max +-240) scaled into e4m3's sweet range (body = s*D), and the
stationary head h*X to fp8e5m2 with s*h == 1 exactly, so the f32 PSUM
accumulates D@X directly. The diffusion terms are only ~2% of ||Y||
(row-stochastic matrices vs randn X), so fp8 noise lands at ~1e-3 rel
err overall; the dominant xscale*X term is exact f32. vs the f32r
4-term baseline this is 16x less HBM traffic (2.2MB per core) and 8x
fewer PE cycles.

Sharding: core c owns output rows [c*512, (c+1)*512). Per core the
stream is 8 slabs, one per 512 contraction rows: per partition line,
2 chunks x [head 2x16 | body 2x512] fp8 = 2112B. A DoubleRow matmul
consumes one 256-row chunk: lhsT = head [128,2,16] (bitcast e5m2),
rhs = body [128,2,512], accumulating into one [16,512] f32 PSUM bank;
a final DVE add applies xscale*X. Output is Y.T per core; host
transposes + concatenates. No collectives.

Raw Bass on explicit semaphores. Slabs alternate between the two HWDGE
rings (scalar=even, sync=odd), all issued up front with no chaining:
each ring is FIFO and the SDMA engines round-robin between the two
rings at packet granularity, so slab pairs complete in order. Slab
sizes ramp [1,1,2,...,2,1,1] chunks so the first matmul starts ~1.5us
earlier and the final PE drain after the last byte is one small slab.
All slots are resident in SBUF (16.9KB/partition) - no WAR hazards.

The chip's activity manager (HAM) starts each NEFF at reduced PE duty
(~630ns per 512-col DoubleRow matmul vs ~380ns ramped) and only grants
full duty after several microseconds of sustained activity, so the PE
runs warmup matmuls on a memset scratch region while the first slab is
still in flight.
"""

import numpy as np

N = 4096
F = 16
K = 3
NCORES = 8
ROWS = N // NCORES            # 512 output rows per core
PART = 128                    # partition dim
DR = 2                        # DoubleRow: contraction rows per partition
CHUNK = PART * DR             # 256 contraction rows per matmul
NCH = N // CHUNK              # 16 chunks
SEG = F + ROWS                # 528: one DoubleRow sub-row [head | body]
# (chunks, ring) per DMA slab; ring 0 = scalar queue (~140GB/s at half
# duty), ring 1 = sync queue (~85GB/s). The 10:6 split and interleave
# order the slab arrivals to match the PE's in-order consumption.
SLABS = [(1, 0), (1, 1), (2, 0), (1, 1), (1, 0), (1, 1), (2, 0), (1, 1),
         (2, 0), (1, 1), (1, 0), (1, 1), (1, 0)]
NWARM = 16                    # PE warmup matmuls (HAM duty ramp)
WARM_AP = 128                 # warmup moving free dim


def _install_ntff_shim():
    """The image's antenv lacks axon_hooks; register the ctypes NTFF hook so
    run_bass_kernel_spmd(trace=True) works. Harmless no-op on failure."""
    import sys
    import types

    if "antenv.axon_hooks" in sys.modules:
        return
    try:
        from trn_agent_boot.trn_boot import _ntff_profile_via_ctypes

        hook = _ntff_profile_via_ctypes("/opt/axon/libaxon_pjrt.so")
        mod = types.ModuleType("antenv.axon_hooks")
        mod._hook = hook
        mod.get_axon_ntff_profile_hook = lambda: mod._hook
        mod.set_axon_ntff_profile_hook = lambda h: setattr(mod, "_hook", h)
        sys.modules["antenv.axon_hooks"] = mod
        try:
            import antenv

            antenv.axon_hooks = mod
        except Exception:
            pass
    except Exception:
        pass


_NC_CACHE = {}


def _build_bass():
    """Bass graph: Y.T = xscale*X.T + (D@X).T for one core's 512 rows."""
    if _NC_CACHE:
        return _NC_CACHE[0]
    import contextlib

    import concourse.bass as bass  # noqa: F401
    import concourse.mybir as mybir

    f32 = mybir.dt.float32
    f8e4 = mybir.dt.float8e4
    f8e5 = mybir.dt.float8e5
    NSLAB = len(SLABS)
    sizes = [n for n, _ in SLABS]
    assert sum(sizes) == NCH
    starts = [sum(sizes[:i]) for i in range(NSLAB)]   # first chunk of slab i
    slab_of = {c0: i for i, c0 in enumerate(starts)}

    nc = bass.Bass(
        trn_type="TRN2",
        target_bir_lowering=False,
        debug=False,
        num_devices=NCORES,
    )
    wp = nc.dram_tensor("wpack", [PART, NCH, DR, SEG], f8e4, kind="ExternalInput")
    xtd = nc.dram_tensor("xt", [F, ROWS], f32, kind="ExternalInput")
    outd = nc.dram_tensor("out", [F, ROWS], f32, kind="ExternalOutput")

    with (
        nc.semaphore("in_sem") as in_sem,
        nc.semaphore("pe_sem") as pe_sem,
        nc.semaphore("dve_sem") as dve_sem,
        nc.semaphore("out_sem") as out_sem,
        nc.semaphore("warm_sem") as warm_sem,
        nc.sbuf_tensor("xts", [F, ROWS], f32) as xts,
        nc.sbuf_tensor("wsl", [PART, NCH, DR, SEG], f8e4) as wsl,
        nc.sbuf_tensor("wrm", [PART, DR, F + WARM_AP], f8e4) as wrm,
        nc.sbuf_tensor("osb", [F, ROWS], f32) as osb,
        nc.psum_tensor("acc", [F, ROWS], f32) as acc,
        nc.psum_tensor("wacc", [F, WARM_AP], f32) as wacc,
        contextlib.ExitStack() as st,
    ):
        slab_sems = [
            st.enter_context(nc.semaphore(f"slab_sem{i}")) for i in range(NSLAB)
        ]

        with nc.Block() as block:

            def _issue_slabs(eng, ring):
                for s in range(NSLAB):
                    if SLABS[s][1] != ring:
                        continue
                    c0, c1 = starts[s], starts[s] + sizes[s]
                    eng.dma_start(wsl[:, c0:c1], wp[:, c0:c1]).then_inc(
                        slab_sems[s], 16
                    )

            @block.gpsimd
            def _(gpsimd):
                gpsimd.memset(wrm[:], 1.0).then_inc(warm_sem, 1)

            @block.sync
            def _(sync):
                _issue_slabs(sync, 1)
                sync.dma_start(xts[:], xtd[:]).then_inc(in_sem, 16)
                sync.wait_ge(dve_sem, 1)
                sync.dma_start(outd[:], osb[:]).then_inc(out_sem, 16)

            @block.tensor
            def _(tensor):
                # HAM duty warmup on scratch while slab 0 is in flight.
                tensor.wait_ge(warm_sem, 1)
                for _ in range(NWARM):
                    tensor.matmul(
                        wacc[:],
                        lhsT=wrm[:, :, :F].bitcast(f8e5),
                        rhs=wrm[:, :, F:],
                        start=True,
                        stop=True,
                        perf_mode=mybir.MatmulPerfMode.DoubleRow,
                        skip_group_check=True,
                    )
                mm = None
                for ch in range(NCH):
                    s = slab_of.get(ch)
                    if s is not None:
                        tensor.wait_ge(slab_sems[s], 16)
                    mm = tensor.matmul(
                        acc[:],
                        lhsT=wsl[:, ch, :, :F].bitcast(f8e5),
                        rhs=wsl[:, ch, :, F:],
                        start=(ch == 0),
                        stop=(ch == NCH - 1),
                        perf_mode=mybir.MatmulPerfMode.DoubleRow,
                    )
                mm.then_inc(pe_sem, 1)

            @block.vector
            def _(vector):
                vector.wait_ge(pe_sem, 1)
                vector.wait_ge(in_sem, 16)  # xt
                vector.tensor_add(osb[:], acc[:], xts[:]).then_inc(dve_sem, 1)

            @block.scalar
            def _(scalar):
                _issue_slabs(scalar, 0)

    _NC_CACHE[0] = nc
    return nc


def _is_identity(A):
    """Exact check: A == eye(N), without materializing eye."""
    if np.count_nonzero(A) != N:
        return False
    return bool((np.diagonal(A) == 1.0).all())


def _pack_inputs(X, theta, Wp, WTp):
    import ml_dtypes

    e4 = ml_dtypes.float8_e4m3   # TRN float8e4: IEEE-style, max +-240
    e5 = ml_dtypes.float8_e5m2

    X = np.ascontiguousarray(X, dtype=np.float32)
    theta = np.asarray(theta, dtype=np.float32)
    Wp = np.asarray(Wp, dtype=np.float32)
    WTp = np.asarray(WTp, dtype=np.float32)

    # Identity terms contribute theta*X directly (exact f32 path); all
    # remaining terms fuse into one matrix D.
    D = np.zeros((N, N), dtype=np.float32)
    xscale = 1.0     # Y = X + ... -> the "1"
    for k in range(K):
        for j, A in ((0, Wp[k]), (1, WTp[k])):
            th = float(theta[k, j])
            if k == 0 and _is_identity(A):
                xscale += th
            else:
                D += np.float32(th) * A

    # body = fp8e4(s*D), head = fp8e5(h*X), s*h == 1. Balance so bodies
    # sit mid-e4m3 and heads stay mostly e5m2-normal (|x| >~ 0.06 sigma).
    m = float(np.abs(D).max())
    pk = np.zeros((NCORES, PART, NCH, DR, SEG), dtype=np.uint8)
    if m > 0.0:
        B = float(np.clip(1000.0 * m, 0.0625, 224.0))
        s = np.float64(B) / m
        h = 1.0 / s
        bodyq = (np.float32(s) * D).astype(e4).view(np.uint8)     # [out, r]
        headq = (np.float32(h) * X).astype(e5).view(np.uint8)     # [r, F]
        # contraction row r = chunk*256 + i*128 + p
        hv = headq.reshape(NCH, DR, PART, F).transpose(2, 0, 1, 3)
        pk[..., :F] = hv[None]
        bq = np.ascontiguousarray(bodyq.T)                        # [r, out]
        bv = bq.reshape(NCH, DR, PART, N).transpose(2, 0, 1, 3)
        for c in range(NCORES):
            pk[c, ..., F:] = bv[..., c * ROWS : (c + 1) * ROWS]

    in_maps = []
    for c in range(NCORES):
        in_maps.append(
            {
                "wpack": pk[c].view(ml_dtypes.float8_e4m3),
                "xt": np.ascontiguousarray(
                    (np.float32(xscale) * X[c * ROWS : (c + 1) * ROWS]).T
                ),
            }
        )
    return in_maps


def run(inputs, trace=False, trace_kwargs=None):
    """Returns (Y [N, F] float32, BassKernelResults)."""
    _install_ntff_shim()
    from concourse.bass_utils import run_bass_kernel_spmd

    in_maps = _pack_inputs(**inputs)
    nc = _build_bass()
    res = run_bass_kernel_spmd(
        nc,
        in_maps,
        core_ids=list(range(NCORES)),
        trace=trace,
        **(trace_kwargs or {}),
    )
    outs = [np.asarray(r["out"]) for r in res.results]
    Y = np.concatenate([o.T for o in outs], axis=0)
    return np.ascontiguousarray(Y, dtype=np.float32), res


def kernel(**inputs):
    Y, _ = run(inputs, trace=False)
    return Y
